# Initial kernel scaffold
#
"""CAM (channel attention module) Trainium2 Bass kernel.

Reference computation (per sample, x: [C, N] with N = H*W):
    energy    = x @ x.T                      # [C, C] Gram matrix
    att       = softmax(rowmax(energy) - energy, axis=-1)
              = softmax(-energy, axis=-1)    # identical after max-shift
    out       = att @ x                      # [C, N]
    result    = gamma * out + x

Sharding: data-parallel over batch, B=16 -> 2 samples per core on 8 cores.

Per-core dataflow (per sample):
  - x [256, 16384] f32 stays resident in SBUF (16 MiB) as 2x8 tiles of
    [128, 2048], loaded once from HBM.
  - Phase 1: PE transposes build [128n, 256c] tiles of x^T on the fly;
    two accumulating fp32r matmuls per n-tile produce energy in PSUM.
  - Softmax: row-min shift (equivalent to the reference's max-shifted
    softmax), exp on ScalarE with fused row-sum; the 1/denom
    normalization is folded into the phase-2 PSUM eviction scale.
  - Phase 2: out = E^T.T @ x with E^T (transposed unnormalized exp
    matrix) stationary; eviction computes gamma/denom * psum + x in one
    VectorE op and streams to HBM.

HBM traffic is the floor: 16 MiB in + 16 MiB out per sample.
"""

import threading

import numpy as np

import concourse.bass as bass
import concourse.mybir as mybir
import concourse.tile as tile
from concourse.bass_utils import run_bass_kernel_spmd
from concourse.masks import make_identity

P = 128
F32 = mybir.dt.float32
F32R = mybir.dt.float32r

# Full-problem shapes (hardcoded per harness contract).
B_FULL = 16
C_FULL = 256
H_FULL = W_FULL = 128
N_CORES = 8
B_PER_CORE = B_FULL // N_CORES  # 2


def emit_cam(tc, x, gamma_b, out, n_s, C, N, xt_cols=2048, chunk=512):
    """Emit the per-core CAM kernel.

    x:       DRAM [n_s, C, N] f32
    gamma_b: DRAM [128, 1] f32 (gamma broadcast to all partitions on host)
    out:     DRAM [n_s, C, N] f32
    """
    nc = tc.nc
    cb_n = C // P            # channel blocks (2)
    nt = N // P              # n-tiles for transposes
    nxt = N // xt_cols       # resident xf tiles per channel block
    nch = N // chunk         # phase-2 output chunks
    assert xt_cols % P == 0 and xt_cols % chunk == 0 and C == 256

    with (
        tc.tile_pool(name="consts", bufs=1) as consts,
        tc.tile_pool(name="xf", bufs=min(2 * nxt + 4, 20)) as xf_pool,
        tc.tile_pool(name="xft", bufs=4) as xft_pool,
        tc.tile_pool(name="att", bufs=4) as att_pool,
        tc.tile_pool(name="attT", bufs=4) as attT_pool,
        tc.tile_pool(name="osb", bufs=6) as osb_pool,
        tc.tile_pool(name="stat", bufs=4) as stat_pool,
        tc.tile_pool(name="eps", bufs=2, space="PSUM") as eps_pool,
        tc.tile_pool(name="ptr", bufs=3, space="PSUM") as ptr_pool,
        tc.tile_pool(name="pout", bufs=3, space="PSUM") as pout_pool,
    ):
        identity = consts.tile([P, P], F32, tag="identity")
        make_identity(nc, identity)
        gamma_sb = consts.tile([P, 1], F32, tag="gamma")
        nc.sync.dma_start(gamma_sb, gamma_b)

        for s in range(n_s):
            # ---- load: x[s] resident as xf[cb][o] tiles [128, xt_cols]
            xf = [[None] * nxt for _ in range(cb_n)]
            for o in range(nxt):
                for cb in range(cb_n):
                    t_ = xf_pool.tile([P, xt_cols], F32, tag="xf",
                                      name=f"xf_s{s}_c{cb}_o{o}")
                    nc.sync.dma_start(
                        t_, x[s, cb * P:(cb + 1) * P, o * xt_cols:(o + 1) * xt_cols])
                    xf[cb][o] = t_

            # ---- phase 1: energy = xf @ xf^T, accumulated over n-tiles
            e_ps = [eps_pool.tile([P, C], F32, tag="eps", name=f"eps_s{s}_m{mb}")
                    for mb in range(cb_n)]
            for t in range(nt):
                o, lc = divmod(t * P, xt_cols)
                ptr = ptr_pool.tile([P, C], F32, tag="ptr", name=f"ptr_s{s}_t{t}")
                for cb in range(cb_n):
                    nc.tensor.transpose(
                        ptr[:, cb * P:(cb + 1) * P], xf[cb][o][:, lc:lc + P], identity)
                xft = xft_pool.tile([P, C], F32, tag="xft", name=f"xft_s{s}_t{t}")
                nc.scalar.copy(xft, ptr)
                for mb in range(cb_n):
                    nc.tensor.matmul(
                        e_ps[mb],
                        lhsT=xft[:, mb * P:(mb + 1) * P].bitcast(F32R),
                        rhs=xft.bitcast(F32R),
                        start=(t == 0), stop=(t == nt - 1))

            # ---- softmax pieces: E = exp(rowmin - energy); denom = rowsum(E)
            att = []
            ginv = []
            for mb in range(cb_n):
                m = stat_pool.tile([P, 1], F32, tag="m", name=f"m_s{s}_{mb}")
                nc.vector.tensor_reduce(
                    m, e_ps[mb], axis=mybir.AxisListType.X, op=mybir.AluOpType.min)
                a = att_pool.tile([P, C], F32, tag="att", name=f"att_s{s}_{mb}")
                den = stat_pool.tile([P, 1], F32, tag="den", name=f"den_s{s}_{mb}")
                nc.scalar.activation(
                    a, e_ps[mb], mybir.ActivationFunctionType.Exp,
                    bias=m, scale=-1.0, accum_out=den)
                inv = stat_pool.tile([P, 1], F32, tag="inv", name=f"inv_s{s}_{mb}")
                nc.vector.reciprocal(inv, den)
                gi = stat_pool.tile([P, 1], F32, tag="gi", name=f"gi_s{s}_{mb}")
                nc.vector.tensor_tensor(gi, inv, gamma_sb, mybir.AluOpType.mult)
                att.append(a)
                ginv.append(gi)

            # E^T tiles (stationary operand of phase 2)
            attT = []
            for jb in range(cb_n):
                ptr2 = ptr_pool.tile([P, C], F32, tag="ptr", name=f"ptrT_s{s}_{jb}")
                for ib in range(cb_n):
                    nc.tensor.transpose(
                        ptr2[:, ib * P:(ib + 1) * P],
                        att[ib][:, jb * P:(jb + 1) * P], identity)
                aT = attT_pool.tile([P, C], F32, tag="attT", name=f"attT_s{s}_{jb}")
                nc.scalar.copy(aT, ptr2)
                attT.append(aT)

            # ---- phase 2: out = gamma/denom * (E^T.T @ xf) + xf
            for ch in range(nch):
                o, lc = divmod(ch * chunk, xt_cols)
                for cb in range(cb_n):
                    po = pout_pool.tile([P, chunk], F32, tag="pout",
                                        name=f"po_s{s}_c{ch}_{cb}")
                    for jb in range(cb_n):
                        nc.tensor.matmul(
                            po,
                            lhsT=attT[jb][:, cb * P:(cb + 1) * P].bitcast(F32R),
                            rhs=xf[jb][o][:, lc:lc + chunk].bitcast(F32R),
                            start=(jb == 0), stop=(jb == cb_n - 1))
                    osb = osb_pool.tile([P, chunk], F32, tag="osb",
                                        name=f"osb_s{s}_c{ch}_{cb}")
                    nc.vector.scalar_tensor_tensor(
                        osb, po, ginv[cb], xf[cb][o][:, lc:lc + chunk],
                        op0=mybir.AluOpType.mult, op1=mybir.AluOpType.add)
                    nc.sync.dma_start(
                        out[s, cb * P:(cb + 1) * P, ch * chunk:(ch + 1) * chunk], osb)


def build_nc(n_s=B_PER_CORE, C=C_FULL, N=H_FULL * W_FULL, xt_cols=2048, chunk=512):
    nc = bass.Bass("TRN2", target_bir_lowering=False, debug=False)
    x = nc.dram_tensor("x", [n_s, C, N], F32, kind="ExternalInput").ap()
    gamma_b = nc.dram_tensor("gamma_b", [P, 1], F32, kind="ExternalInput").ap()
    out = nc.dram_tensor("out", [n_s, C, N], F32, kind="ExternalOutput").ap()
    with tile.TileContext(nc) as tc:
        emit_cam(tc, x, gamma_b, out, n_s, C, N, xt_cols=xt_cols, chunk=chunk)
    return nc


_CACHE = threading.Lock()
_NC = None


def _get_nc():
    global _NC
    with _CACHE:
        if _NC is None:
            _NC = build_nc()
    return _NC


def run_spmd(x, gamma, **kwargs):
    """Shard inputs over 8 cores, run, gather. Returns (output, BassKernelResults)."""
    x = np.ascontiguousarray(np.asarray(x), dtype=np.float32)
    assert x.shape == (B_FULL, C_FULL, H_FULL, W_FULL), x.shape
    n = H_FULL * W_FULL
    xs = x.reshape(B_FULL, C_FULL, n)
    gb = np.full((P, 1), np.float32(np.asarray(gamma)), dtype=np.float32)
    in_maps = [
        {"x": xs[c * B_PER_CORE:(c + 1) * B_PER_CORE], "gamma_b": gb}
        for c in range(N_CORES)
    ]
    nc = _get_nc()
    res = run_bass_kernel_spmd(nc, in_maps, core_ids=list(range(N_CORES)), **kwargs)
    outs = np.stack([res.results[c]["out"] for c in range(N_CORES)])
    full = outs.reshape(B_FULL, C_FULL, H_FULL, W_FULL).astype(np.float32, copy=False)
    return full, res


def kernel(x, gamma):
    out, _ = run_spmd(x, gamma)
    return out


# revision 15
# speedup vs baseline: 1.3037x; 1.3037x over previous
"""CAM (channel attention module) Trainium2 Bass kernel.

Reference computation (per sample, x: [C, N] with N = H*W):
    energy    = x @ x.T                      # [C, C] Gram matrix
    att       = softmax(rowmax(energy) - energy, axis=-1)
              = softmax(-energy, axis=-1)    # identical after max-shift
    out       = att @ x                      # [C, N]
    result    = gamma * out + x

Sharding: data-parallel over batch, B=16 -> 2 samples per core on 8 cores.

Per-core dataflow (per sample):
  - x [256, 16384] f32 stays resident in SBUF (16 MiB) as 2x8 tiles of
    [128, 2048], loaded once from HBM.
  - Phase 1: PE transposes build [128n, 256c] tiles of x^T on the fly;
    two accumulating fp32r matmuls per n-tile produce energy in PSUM.
  - Softmax: row-min shift (equivalent to the reference's max-shifted
    softmax), exp on ScalarE with fused row-sum; the 1/denom
    normalization is folded into the phase-2 PSUM eviction scale.
  - Phase 2: out = E^T.T @ x with E^T (transposed unnormalized exp
    matrix) stationary; eviction computes gamma/denom * psum + x in one
    VectorE op and streams to HBM.

HBM traffic is the floor: 16 MiB in + 16 MiB out per sample.
"""

import threading

import numpy as np

import concourse.bass as bass
import concourse.mybir as mybir
import concourse.tile as tile
from concourse import bacc
from concourse.bass_utils import run_bass_kernel_spmd
from concourse.masks import make_identity

P = 128
F32 = mybir.dt.float32
F32R = mybir.dt.float32r

# Full-problem shapes (hardcoded per harness contract).
B_FULL = 16
C_FULL = 256
H_FULL = W_FULL = 128
N_CORES = 8
B_PER_CORE = B_FULL // N_CORES  # 2


def emit_cam(tc, x, gamma_b, out, n_s, C, N, xt_cols=2048, chunk=512,
             xf_bufs=None, osb_bufs=6, xfr_bufs=6, ptr_bufs=3, pout_bufs=3,
             xft_bufs=4):
    """Emit the per-core CAM kernel.

    x:       DRAM [n_s, C, N] f32
    gamma_b: DRAM [128, 1] f32 (gamma broadcast to all partitions on host)
    out:     DRAM [n_s, C, N] f32
    """
    nc = tc.nc
    cb_n = C // P            # channel blocks (2)
    nt = N // P              # n-tiles for transposes
    nxt = N // xt_cols       # resident xf tiles per channel block
    nch = N // chunk         # phase-2 output chunks
    assert xt_cols % P == 0 and xt_cols % chunk == 0 and C == 256

    if xf_bufs is None:
        xf_bufs = 2 * nxt + 2
    with (
        tc.tile_pool(name="consts", bufs=1) as consts,
        tc.tile_pool(name="xf", bufs=xf_bufs) as xf_pool,
        tc.tile_pool(name="xft", bufs=xft_bufs) as xft_pool,
        tc.tile_pool(name="att", bufs=4) as att_pool,
        tc.tile_pool(name="attT", bufs=4) as attT_pool,
        tc.tile_pool(name="osb", bufs=osb_bufs) as osb_pool,
        tc.tile_pool(name="xfr", bufs=xfr_bufs) as xfr_pool,
        tc.tile_pool(name="stat", bufs=4) as stat_pool,
        tc.tile_pool(name="eps", bufs=2, space="PSUM") as eps_pool,
        tc.tile_pool(name="ptr", bufs=ptr_bufs, space="PSUM") as ptr_pool,
        tc.tile_pool(name="pout", bufs=pout_bufs, space="PSUM") as pout_pool,
    ):
        identity = consts.tile([P, P], F32, tag="identity")
        make_identity(nc, identity)
        gamma_sb = consts.tile([P, 1], F32, tag="gamma")
        nc.sync.dma_start(gamma_sb, gamma_b)

        for s in range(n_s):
            # ---- load: x[s] resident as xf[cb][o] tiles [128, xt_cols]
            # Tiles stay f32: a float32r-typed DMA destination makes the DGE
            # round the payload to fp32r precision in flight, which would
            # corrupt the exact residual copy of x. fp32r views are taken
            # only at matmul operands.
            xf = [[None] * nxt for _ in range(cb_n)]
            for o in range(nxt):
                for cb in range(cb_n):
                    t_ = xf_pool.tile([P, xt_cols], F32, tag="xf",
                                      name=f"xf_s{s}_c{cb}_o{o}")
                    nc.sync.dma_start(
                        t_, x[s, cb * P:(cb + 1) * P, o * xt_cols:(o + 1) * xt_cols])
                    xf[cb][o] = t_

            # ---- phase 1: energy = xf @ xf^T, accumulated over n-tiles
            e_ps = [eps_pool.tile([P, C], F32, tag="eps", name=f"eps_s{s}_m{mb}")
                    for mb in range(cb_n)]
            for t in range(nt):
                o, lc = divmod(t * P, xt_cols)
                ptr = ptr_pool.tile([P, C], F32, tag="ptr", name=f"ptr_s{s}_t{t}")
                for cb in range(cb_n):
                    nc.tensor.transpose(
                        ptr[:, cb * P:(cb + 1) * P],
                        xf[cb][o][:, lc:lc + P], identity)
                xft = xft_pool.tile([P, C], F32R, tag="xft", name=f"xft_s{s}_t{t}")
                nc.scalar.copy(xft, ptr)
                for mb in range(cb_n):
                    nc.tensor.matmul(
                        e_ps[mb],
                        lhsT=xft[:, mb * P:(mb + 1) * P],
                        rhs=xft,
                        start=(t == 0), stop=(t == nt - 1))

            # ---- softmax pieces: E = exp(rowmin - energy); denom = rowsum(E)
            att = []
            ginv = []
            for mb in range(cb_n):
                m = stat_pool.tile([P, 1], F32, tag="m", name=f"m_s{s}_{mb}")
                nc.vector.tensor_reduce(
                    m, e_ps[mb], axis=mybir.AxisListType.X, op=mybir.AluOpType.min)
                a = att_pool.tile([P, C], F32, tag="att", name=f"att_s{s}_{mb}")
                den = stat_pool.tile([P, 1], F32, tag="den", name=f"den_s{s}_{mb}")
                nc.scalar.activation(
                    a, e_ps[mb], mybir.ActivationFunctionType.Exp,
                    bias=m, scale=-1.0, accum_out=den)
                inv = stat_pool.tile([P, 1], F32, tag="inv", name=f"inv_s{s}_{mb}")
                nc.vector.reciprocal(inv, den)
                gi = stat_pool.tile([P, 1], F32, tag="gi", name=f"gi_s{s}_{mb}")
                nc.vector.tensor_tensor(gi, inv, gamma_sb, mybir.AluOpType.mult)
                att.append(a)
                ginv.append(gi)

            # E^T tiles (stationary operand of phase 2)
            attT = []
            for jb in range(cb_n):
                ptr2 = ptr_pool.tile([P, C], F32, tag="ptr", name=f"ptrT_s{s}_{jb}")
                for ib in range(cb_n):
                    nc.tensor.transpose(
                        ptr2[:, ib * P:(ib + 1) * P],
                        att[ib][:, jb * P:(jb + 1) * P], identity)
                aT = attT_pool.tile([P, C], F32R, tag="attT", name=f"attT_s{s}_{jb}")
                nc.scalar.copy(aT, ptr2)
                attT.append(aT)

            # ---- phase 2: out = gamma/denom * (E^T.T @ xf) + xf
            for ch in range(nch):
                o, lc = divmod(ch * chunk, xt_cols)
                # fp32r-rounded copy of this chunk (walrus requires fp32r
                # matmul operands to come from a rounding instruction; the
                # rounded copy is reused by both output channel blocks).
                xfr = []
                for jb in range(cb_n):
                    r_ = xfr_pool.tile([P, chunk], F32R, tag="xfr",
                                       name=f"xfr_s{s}_c{ch}_{jb}")
                    nc.vector.tensor_copy(r_, xf[jb][o][:, lc:lc + chunk])
                    xfr.append(r_)
                for cb in range(cb_n):
                    po = pout_pool.tile([P, chunk], F32, tag="pout",
                                        name=f"po_s{s}_c{ch}_{cb}")
                    for jb in range(cb_n):
                        nc.tensor.matmul(
                            po,
                            lhsT=attT[jb][:, cb * P:(cb + 1) * P],
                            rhs=xfr[jb],
                            start=(jb == 0), stop=(jb == cb_n - 1))
                    osb = osb_pool.tile([P, chunk], F32, tag="osb",
                                        name=f"osb_s{s}_c{ch}_{cb}")
                    nc.vector.scalar_tensor_tensor(
                        osb, po, ginv[cb], xf[cb][o][:, lc:lc + chunk],
                        op0=mybir.AluOpType.mult, op1=mybir.AluOpType.add)
                    nc.sync.dma_start(
                        out[s, cb * P:(cb + 1) * P, ch * chunk:(ch + 1) * chunk], osb)


def build_nc(n_s=B_PER_CORE, C=C_FULL, N=H_FULL * W_FULL, xt_cols=2048, chunk=512):
    nc = bacc.Bacc("TRN2", target_bir_lowering=False, debug=False)
    x = nc.dram_tensor("x", [n_s, C, N], F32, kind="ExternalInput").ap()
    gamma_b = nc.dram_tensor("gamma_b", [P, 1], F32, kind="ExternalInput").ap()
    out = nc.dram_tensor("out", [n_s, C, N], F32, kind="ExternalOutput").ap()
    with tile.TileContext(nc) as tc:
        emit_cam(tc, x, gamma_b, out, n_s, C, N, xt_cols=xt_cols, chunk=chunk)
    nc.compile()
    return nc


_CACHE = threading.Lock()
_NC = None


def _get_nc():
    global _NC
    with _CACHE:
        if _NC is None:
            _NC = build_nc()
    return _NC


def run_spmd(x, gamma, **kwargs):
    """Shard inputs over 8 cores, run, gather. Returns (output, BassKernelResults)."""
    x = np.ascontiguousarray(np.asarray(x), dtype=np.float32)
    assert x.shape == (B_FULL, C_FULL, H_FULL, W_FULL), x.shape
    n = H_FULL * W_FULL
    xs = x.reshape(B_FULL, C_FULL, n)
    gb = np.full((P, 1), np.float32(np.asarray(gamma)), dtype=np.float32)
    in_maps = [
        {"x": xs[c * B_PER_CORE:(c + 1) * B_PER_CORE], "gamma_b": gb}
        for c in range(N_CORES)
    ]
    nc = _get_nc()
    res = run_bass_kernel_spmd(nc, in_maps, core_ids=list(range(N_CORES)), **kwargs)
    outs = np.stack([res.results[c]["out"] for c in range(N_CORES)])
    full = outs.reshape(B_FULL, C_FULL, H_FULL, W_FULL).astype(np.float32, copy=False)
    return full, res


def kernel(x, gamma):
    out, _ = run_spmd(x, gamma)
    return out
